# revision 19
# baseline (speedup 1.0000x reference)
"""Causal self-attention (B=4, T=2048, D=1024, H=16) on 8 TRN2 NeuronCores.

Sharding: tensor-parallel over heads. Each core owns 2 heads: it computes
Q/K/V projections for its head-slice of W_qkv (column-parallel), full causal
attention for those heads, and a partial output projection with its row-slice
of W_out (row-parallel). The host sums the 8 partials and adds b_out.

Per-core kernel layout (all matmuls bf16 with fp32 PSUM accumulation):
  - x is pre-transposed on the host to xT [D, B*T] so the projection
    contraction dim (D) lies on SBUF partitions.
  - Projections produce Q^T/K^T [n, t] directly (W chunks stationary,
    xT chunks moving); V is produced as V^T then PE-transposed to [t, dv].
  - Scores are computed transposed, S^T [keys, q], two heads packed into
    one PSUM tile via row-group tiling (contraction dim is 64 per head).
  - Softmax skips the max subtraction (scores are O(1) by construction:
    exp never overflows), so exp comes straight off PSUM via ScalarE.
  - The AV matmul's stationary operand is [V_h | ones*64] (128 cols), so
    partitions 64:128 of the O accumulator hold the softmax denominator
    replicated 64x - normalization is one reciprocal + one multiply.
  - Causality: key-chunk matmuls on the diagonal are narrowed to the
    valid query range; the 128x128 boundary subtile is masked with a
    triangular constant after exp.
  - The attention inner loop is ACT(exp)-bound, so independent PE work
    (next batch's projection + V transposes, previous block's output
    projection) is interleaved into it via filler generators to keep the
    TensorEngine dense (and the HAM clock-gate warm).
"""
import os
import numpy as np
import ml_dtypes
from contextlib import ExitStack

import concourse.bass as bass
import concourse.tile as tile
from concourse import bacc, mybir
from concourse.bass_utils import run_bass_kernel_spmd

# This kernel's only ACT functions are Exp and Ln, which share the
# natural_log_exp_and_others table set. By default the table-load pass maps
# Exp to the earlier exp_and_others set, thrashing two ~1.3us table reloads
# around every Ln pair. Narrow the pass's view so Exp resolves only to the
# shared set (list order is preserved, so emitted act_func_set_ids stay
# valid act_info.json indices).
_orig_gat = bacc.get_activation_tables


def _gat_ln_exp(arch):
    out = {}
    for name, funcs in _orig_gat(arch).items():
        if name != "natural_log_exp_and_others":
            funcs = funcs - {mybir.ActivationFunctionType.Exp}
        out[name] = funcs
    return out


bacc.get_activation_tables = _gat_ln_exp

N_CORES = 8
B, T, D = 4, 2048, 1024
H, DH = 16, 64
HPC = H // N_CORES          # heads per core = 2
BT = B * T                  # 8192
TPB = T // 512              # 4 token blocks per batch
NKC = T // 128              # 16 key chunks per batch
NQB = T // 512              # 4 query blocks per batch

F32 = mybir.dt.float32
BF16 = mybir.dt.bfloat16
EXPF = mybir.ActivationFunctionType.Exp

_CACHED_NC = None
LAST_RESULTS = None  # test harness reads exec_time from here


def _act_recip(nc, out, in_, scratch):
    """1/x on ScalarE as exp(-ln(x)). Ln and Exp share one ACT table set
    (natural_log_exp_and_others) so this costs no table reloads, unlike the
    Reciprocal table (2 reloads per use, ~2.6us). Roundtrip rel err ~1e-6
    for the softmax-denominator range, far below this kernel's bf16 noise
    floor; ~4.5x cheaper than the DVE iterative divide and runs on the
    less-loaded engine."""
    nc.scalar.activation(scratch, in_, mybir.ActivationFunctionType.Ln)
    nc.scalar.activation(out, scratch, EXPF, scale=-1.0)


def _build():
    nc = bacc.Bacc("TRN2", target_bir_lowering=False, debug=False,
                   num_devices=N_CORES)
    d_xT = nc.dram_tensor("xT", [D, BT], BF16, kind="ExternalInput").ap()
    d_wq = nc.dram_tensor("wq", [D, 128], BF16, kind="ExternalInput").ap()
    d_wk = nc.dram_tensor("wk", [D, 128], BF16, kind="ExternalInput").ap()
    d_wv = nc.dram_tensor("wv", [D, 128], BF16, kind="ExternalInput").ap()
    d_wo = nc.dram_tensor("wo", [128, D], BF16, kind="ExternalInput").ap()
    d_bias = nc.dram_tensor("bias", [128, 3], F32, kind="ExternalInput").ap()
    d_tri = nc.dram_tensor("tri", [128, 128], BF16, kind="ExternalInput").ap()
    d_ident = nc.dram_tensor("ident", [128, 128], BF16, kind="ExternalInput").ap()
    d_out = nc.dram_tensor("out", [BT, D], F32, kind="ExternalOutput").ap()

    with tile.TileContext(nc) as tc:
        with ExitStack() as ctx:
            consts = ctx.enter_context(tc.tile_pool(name="consts", bufs=1))
            big = ctx.enter_context(tc.tile_pool(name="big", bufs=1))
            vtpool = ctx.enter_context(tc.tile_pool(name="vt", bufs=2))
            xpool = ctx.enter_context(tc.tile_pool(name="xt", bufs=3))
            ppool = ctx.enter_context(tc.tile_pool(name="pt", bufs=6))
            opool = ctx.enter_context(tc.tile_pool(name="ot", bufs=12))
            rpool = ctx.enter_context(tc.tile_pool(name="rc", bufs=2))
            outp = ctx.enter_context(tc.tile_pool(name="outp", bufs=6))
            psA = ctx.enter_context(tc.tile_pool(name="psA", bufs=2, space="PSUM"))
            psO = ctx.enter_context(tc.tile_pool(name="psO", bufs=2, space="PSUM"))
            psM = ctx.enter_context(tc.tile_pool(name="psM", bufs=1, space="PSUM"))

            # ---- constants ----
            wq_sb = consts.tile([128, 1024], BF16, tag="wq")
            wk_sb = consts.tile([128, 1024], BF16, tag="wk")
            wv_sb = consts.tile([128, 1024], BF16, tag="wv")
            # (c p) n -> p (c n): k-chunk c of W lands at cols [c*128, c*128+128)
            for w_sb, d_w in ((wq_sb, d_wq), (wk_sb, d_wk), (wv_sb, d_wv)):
                nc.sync.dma_start(
                    w_sb[:].rearrange("p (c n) -> p c n", c=8),
                    d_w.rearrange("(c p) n -> p c n", p=128))
            wo_sb = consts.tile([128, 1024], BF16, tag="wo")
            nc.sync.dma_start(wo_sb[:], d_wo[:])
            bias_sb = consts.tile([128, 3], F32, tag="bias")
            nc.sync.dma_start(bias_sb[:], d_bias[:])
            tri_sb = consts.tile([128, 128], BF16, tag="tri")
            nc.sync.dma_start(tri_sb[:], d_tri[:])
            ident_sb = consts.tile([128, 128], BF16, tag="ident")
            nc.sync.dma_start(ident_sb[:], d_ident[:])

            # ---- persistent per-batch tensors ----
            qt = [big.tile([128, T], BF16, tag=f"qt{b}", name=f"qt{b}")
                  for b in range(B)]
            kt = [big.tile([128, T], BF16, tag=f"kt{b}", name=f"kt{b}")
                  for b in range(B)]
            # v_sb[b]: 16 key chunks x [V_h0 | ones | V_h1 | ones] (256 cols)
            v_sb = [big.tile([128, NKC * 256], BF16, tag=f"v{b}", name=f"v{b}")
                    for b in range(B)]
            for b in range(B):
                nc.gpsimd.memset(v_sb[b][:], 1.0)

            vt_tmp = [None] * B   # V^T staging per batch
            proj_prog = [0] * B   # completed t-blocks per batch
            proj_cap = [TPB] * B  # phase throttle: max t-blocks to emit

            def proj_gen(b):
                """Projection + V transpose of one t-block at a time, in
                small PE steps. attn(b, qb) only reads K/V key chunks up to
                t-block qb, so later t-blocks legally interleave INTO batch
                b's own attention - the filler that keeps the last batch's
                TensorEngine dense (and its HAM clock warm)."""
                vt_tmp[b] = vtpool.tile([128, T], BF16, tag="vt", name=f"vt{b}")
                for tbl in range(TPB):
                    x_t = xpool.tile([128, 8 * 512], BF16, tag="xt",
                                     name=f"x{b}_{tbl}")
                    nc.sync.dma_start(
                        x_t[:].rearrange("p (c t) -> p c t", c=8),
                        d_xT[:, bass.ts(b * TPB + tbl, 512)]
                            .rearrange("(c p) t -> p c t", p=128))
                    for pi, (w_sb, col) in enumerate(
                            ((wq_sb, 0), (wk_sb, 1), (wv_sb, 2))):
                        # alternate pools: double-buffers the accumulator
                        # across groups despite each pool having bufs=1
                        gpool, gtag = ((psM, "proj"), (psO, "proj2"))[
                            (tbl * 3 + pi) % 2]
                        ps = gpool.tile([128, 512], F32, tag=gtag, bufs=1,
                                        name=f"pj{b}_{tbl}_{pi}")
                        for c in range(8):
                            nc.tensor.matmul(
                                ps[:], w_sb[:, bass.ts(c, 128)],
                                x_t[:, bass.ts(c, 512)],
                                start=(c == 0), stop=(c == 7))
                            yield
                        dest = (qt[b], kt[b], vt_tmp[b])[pi]
                        nc.vector.tensor_scalar_add(
                            dest[:, bass.ts(tbl, 512)], ps[:],
                            bias_sb[:, col:col + 1])
                    # this t-block's V^T -> v_sb [t, (V|1|V|1)]: 4 PE
                    # transposes staged in the 1-bank psM pool (bitcast)
                    tp = psM.tile([128, 512], F32, tag="proj",
                                  name=f"tp{b}_{tbl}")
                    tpb = tp[:].bitcast(BF16)
                    for t4 in range(4):
                        tc16 = tbl * 4 + t4
                        nc.tensor.transpose(
                            tpb[:, t4 * 128: t4 * 128 + 128],
                            vt_tmp[b][:, bass.ts(tc16, 128)], ident_sb[:])
                    src = bass.AP(tpb.tensor, tpb.offset,
                                  [tpb.ap[0], [128, 4], [64, 2], [1, 64]])
                    dst0 = v_sb[b][:, tbl * 1024: tbl * 1024 + 1024]
                    dst = bass.AP(dst0.tensor, dst0.offset,
                                  [dst0.ap[0], [256, 4], [128, 2], [1, 64]])
                    nc.vector.tensor_copy(dst, src)
                    proj_prog[b] = tbl + 1
                    yield

            def outproj_gen(b, qb, o_sb):
                """out[q, n] = sum_dv O^T[dv, q] * W_out[dv, n], per q-chunk."""
                for qc in range(4):
                    op = psA.tile([128, 1024], F32, tag="sA",
                                  name=f"op{b}_{qb}_{qc}")
                    for n2 in range(2):
                        nc.tensor.matmul(
                            op[:, bass.ts(n2, 512)],
                            o_sb[:, bass.ts(qc, 128)],
                            wo_sb[:, bass.ts(n2, 512)],
                            start=True, stop=True)
                        yield
                    osb = outp.tile([128, 1024], F32, tag="outp",
                                    name=f"ob{b}_{qb}_{qc}")
                    nc.vector.tensor_copy(osb[:], op[:])
                    row = b * T + qb * 512 + qc * 128
                    nc.sync.dma_start(d_out[row:row + 128, :], osb[:])
                    yield

            fill_proj = []  # (batch, generator) projection fillers
            fill_op = []    # short out-projection generators
            fill_kw = []    # keep-warm dummy matmuls (lowest priority)

            def kw_gen(n):
                for i in range(n):
                    kw = psM.tile([128, 512], F32, tag="proj", name=f"kw{i}")
                    nc.tensor.matmul(kw[:], wo_sb[:, 0:128], wo_sb[:, 0:512],
                                     start=True, stop=True)
                    yield

            def pull_from(lst, n):
                for _ in range(n):
                    while lst:
                        try:
                            next(lst[0])
                            break
                        except StopIteration:
                            lst.pop(0)
                    else:
                        break

            def pull_proj(n):
                got = 0
                for _ in range(n):
                    while fill_proj:
                        pb, g = fill_proj[0]
                        if proj_prog[pb] >= proj_cap[pb]:
                            return got  # head gen throttled for a later phase
                        try:
                            next(g)
                            got += 1
                            break
                        except StopIteration:
                            fill_proj.pop(0)
                    else:
                        break
                return got

            def force_proj(b, upto):
                """Emit batch b's projection through t-block `upto` NOW
                (earlier batches' leftovers drain first - they are older
                dependencies by construction)."""
                proj_cap[b] = max(proj_cap[b], upto)
                while proj_prog[b] < upto and fill_proj:
                    pb, g = fill_proj[0]
                    try:
                        next(g)
                    except StopIteration:
                        fill_proj.pop(0)

            def pull(n):
                # out-projections are short and slot-critical: keep them moving
                pull_from(fill_op, 1)
                if pull_proj(n) == 0 and not fill_op:
                    # real filler dry (last batch): burn one cheap matmul into
                    # the idle projection PSUM bank so the HAM activity
                    # monitor keeps the PE clock at 2.4GHz - the dummy work
                    # costs ~0.2us but cold-clocking the remaining real
                    # matmuls costs ~2x their runtime
                    pull_from(fill_kw, 1)

            def attn(b, qb):
                """Attention for query block qb of batch b."""
                o_ps = [psO.tile([128, 512], F32, tag="o",
                                 name=f"ops{b}_{qb}_{h}") for h in range(2)]
                nch = 4 * qb + 4
                pending = None  # (p_t, off, j) awaiting AV matmuls

                def av(p_t, off, j):
                    for h in range(2):
                        lo = off if h == 0 else 512
                        nc.tensor.matmul(
                            o_ps[h][:, off:512],
                            v_sb[b][:, j * 256 + h * 128: j * 256 + h * 128 + 128],
                            p_t[:, lo: lo + 512 - off],
                            start=(j == 0), stop=(j == nch - 1))

                for j in range(nch):
                    r = j - 4 * qb
                    off = 128 * r if r >= 0 else 0
                    s_ps = psA.tile([128, 1024], F32, tag="sA",
                                    name=f"s{b}_{qb}_{j}")
                    p_t = ppool.tile([128, 1024], BF16, tag="pt",
                                     name=f"p{b}_{qb}_{j}")
                    # h0's valid q-range lands at [off:512], h1's at
                    # [512:1024-off]: adjacent, so one exp covers both heads
                    for h in range(2):
                        lo = off if h == 0 else 512
                        nc.tensor.matmul(
                            s_ps[:, lo: lo + 512 - off],
                            kt[b][64 * h: 64 * h + 64, bass.ts(j, 128)],
                            qt[b][64 * h: 64 * h + 64,
                                  qb * 512 + off: qb * 512 + 512],
                            start=True, stop=True, tile_position=(64 * h, 0))
                    nc.scalar.activation(p_t[:, off: 1024 - off],
                                         s_ps[:, off: 1024 - off],
                                         EXPF, scale=0.125)
                    if r >= 0:
                        for h in range(2):
                            lo = off if h == 0 else 512
                            nc.vector.tensor_mul(
                                p_t[:, lo: lo + 128],
                                p_t[:, lo: lo + 128],
                                tri_sb[:])
                    if pending is not None:
                        av(*pending)
                    pending = (p_t, off, j)
                    pull(2)
                av(*pending)
                # bound the out-projection backlog so the o_sb slot chain
                # below can't deadlock (opool bufs exceeds backlog + 1);
                # the backlog doubles as PE filler for the last batch,
                # which has no projection work left to interleave - but
                # taper it off through that batch so nothing piles into a
                # serial drain after the last attention block
                # out-projection backlog: batch 0 drains promptly; batches
                # 1-2 defer (reserve PE work for the filler-poor tail);
                # batch 3 spends the reserve across its blocks, ending dry
                if b == 0:
                    limit = 5
                elif b < 3:
                    limit = 99
                else:
                    limit = max(0, 6 - 2 * qb)
                while len(fill_op) > limit:
                    pull_from(fill_op, 10 ** 9)

                # normalize: O[dv, q] / denom[q] (denom replicated on 64:128)
                o_sb = opool.tile([128, 512], BF16, tag="ot",
                                  name=f"o{b}_{qb}")
                for h in range(2):
                    lg = rpool.tile([64, 512], F32, tag="lg",
                                    name=f"lg{b}_{qb}_{h}")
                    rec = rpool.tile([64, 512], F32, tag="rc",
                                     name=f"r{b}_{qb}_{h}")
                    _act_recip(nc, rec[:], o_ps[h][64:128, :], lg[:])
                    nc.vector.tensor_mul(
                        o_sb[64 * h: 64 * h + 64, :], o_ps[h][0:64, :], rec[:])
                fill_op.append(outproj_gen(b, qb, o_sb))

            # ---- emission ----
            fill_proj.append((0, proj_gen(0)))
            force_proj(0, TPB)
            for b in range(B):
                if b + 1 < B:
                    # reserve the last batch's later t-blocks as filler for
                    # its own attention; earlier batches emit fully as filler
                    # of their predecessor
                    proj_cap[b + 1] = 1 if b + 1 == B - 1 else TPB
                    fill_proj.append((b + 1, proj_gen(b + 1)))
                if b == B - 1:
                    proj_cap[b] = TPB
                    fill_kw.append(kw_gen(80))
                for qb in range(NQB):
                    # attention of block qb reads K/V only up to t-block qb
                    force_proj(b, min(qb + 1, TPB))
                    attn(b, qb)
            pull_from(fill_op, 10 ** 9)

    nc.compile()
    return nc


def _prep_inputs(x, W_qkv, b_qkv, W_out):
    bf = ml_dtypes.bfloat16
    flat = np.ascontiguousarray(x.reshape(BT, D))
    xT = np.ascontiguousarray(flat.T).astype(bf)
    tri = np.triu(np.ones((128, 128), np.float32)).astype(bf)
    ident = np.eye(128, dtype=np.float32).astype(bf)
    in_maps = []
    for c in range(N_CORES):
        sl = slice(128 * c, 128 * c + 128)
        in_maps.append({
            "xT": xT,
            "wq": np.ascontiguousarray(W_qkv[:, 0 * D:1 * D][:, sl]).astype(bf),
            "wk": np.ascontiguousarray(W_qkv[:, 1 * D:2 * D][:, sl]).astype(bf),
            "wv": np.ascontiguousarray(W_qkv[:, 2 * D:3 * D][:, sl]).astype(bf),
            "wo": np.ascontiguousarray(W_out[sl, :]).astype(bf),
            "bias": np.ascontiguousarray(np.stack(
                [b_qkv[0 * D:1 * D][sl], b_qkv[1 * D:2 * D][sl],
                 b_qkv[2 * D:3 * D][sl]], axis=1)).astype(np.float32),
            "tri": tri,
            "ident": ident,
        })
    return in_maps


def kernel(x, W_qkv, b_qkv, W_out, b_out):
    global _CACHED_NC, LAST_RESULTS
    x = np.asarray(x, np.float32)
    W_qkv = np.asarray(W_qkv, np.float32)
    b_qkv = np.asarray(b_qkv, np.float32)
    W_out = np.asarray(W_out, np.float32)
    b_out = np.asarray(b_out, np.float32)

    if _CACHED_NC is None:
        _CACHED_NC = _build()
    in_maps = _prep_inputs(x, W_qkv, b_qkv, W_out)
    res = run_bass_kernel_spmd(
        _CACHED_NC, in_maps, core_ids=list(range(N_CORES)),
        trace=bool(int(os.environ.get("ATTN_TRACE", "0"))))
    LAST_RESULTS = res
    acc = np.zeros((BT, D), np.float64)
    for r in res.results:
        acc += r["out"].astype(np.float64)
    out = (acc + b_out.astype(np.float64)).astype(np.float32)
    return out.reshape(B, T, D)


# revision 20
# speedup vs baseline: 1.0127x; 1.0127x over previous
"""Causal self-attention (B=4, T=2048, D=1024, H=16) on 8 TRN2 NeuronCores.

Sharding: tensor-parallel over heads. Each core owns 2 heads: it computes
Q/K/V projections for its head-slice of W_qkv (column-parallel), full causal
attention for those heads, and a partial output projection with its row-slice
of W_out (row-parallel). The host sums the 8 partials and adds b_out.

Per-core kernel layout (all matmuls bf16 with fp32 PSUM accumulation):
  - x is pre-transposed on the host to xT [D, B*T] so the projection
    contraction dim (D) lies on SBUF partitions.
  - Projections produce Q^T/K^T [n, t] directly (W chunks stationary,
    xT chunks moving); V is produced as V^T then PE-transposed to [t, dv].
  - Scores are computed transposed, S^T [keys, q], two heads packed into
    one PSUM tile via row-group tiling (contraction dim is 64 per head).
  - Softmax skips the max subtraction (scores are O(1) by construction:
    exp never overflows), so exp comes straight off PSUM via ScalarE.
  - The AV matmul's stationary operand is [V_h | ones*64] (128 cols), so
    partitions 64:128 of the O accumulator hold the softmax denominator
    replicated 64x - normalization is one reciprocal + one multiply.
  - Causality: key-chunk matmuls on the diagonal are narrowed to the
    valid query range; the 128x128 boundary subtile is masked with a
    triangular constant after exp.
  - The attention inner loop is ACT(exp)-bound, so independent PE work
    (next batch's projection + V transposes, previous block's output
    projection) is interleaved into it via filler generators to keep the
    TensorEngine dense (and the HAM clock-gate warm).
"""
import os
import numpy as np
import ml_dtypes
from contextlib import ExitStack

import concourse.bass as bass
import concourse.tile as tile
from concourse import bacc, mybir
from concourse.bass_utils import run_bass_kernel_spmd

# This kernel's only ACT functions are Exp and Ln, which share the
# natural_log_exp_and_others table set. By default the table-load pass maps
# Exp to the earlier exp_and_others set, thrashing two ~1.3us table reloads
# around every Ln pair. Narrow the pass's view so Exp resolves only to the
# shared set (list order is preserved, so emitted act_func_set_ids stay
# valid act_info.json indices).
_orig_gat = bacc.get_activation_tables


def _gat_ln_exp(arch):
    out = {}
    for name, funcs in _orig_gat(arch).items():
        if name != "natural_log_exp_and_others":
            funcs = funcs - {mybir.ActivationFunctionType.Exp}
        out[name] = funcs
    return out


bacc.get_activation_tables = _gat_ln_exp

N_CORES = 8
B, T, D = 4, 2048, 1024
H, DH = 16, 64
HPC = H // N_CORES          # heads per core = 2
BT = B * T                  # 8192
TPB = T // 512              # 4 token blocks per batch
NKC = T // 128              # 16 key chunks per batch
NQB = T // 512              # 4 query blocks per batch

F32 = mybir.dt.float32
BF16 = mybir.dt.bfloat16
EXPF = mybir.ActivationFunctionType.Exp

_CACHED_NC = None
LAST_RESULTS = None  # test harness reads exec_time from here


def _act_recip(nc, out, in_, scratch):
    """1/x on ScalarE as exp(-ln(x)). Ln and Exp share one ACT table set
    (natural_log_exp_and_others) so this costs no table reloads, unlike the
    Reciprocal table (2 reloads per use, ~2.6us). Roundtrip rel err ~1e-6
    for the softmax-denominator range, far below this kernel's bf16 noise
    floor; ~4.5x cheaper than the DVE iterative divide and runs on the
    less-loaded engine."""
    nc.scalar.activation(scratch, in_, mybir.ActivationFunctionType.Ln)
    nc.scalar.activation(out, scratch, EXPF, scale=-1.0)


def _build():
    nc = bacc.Bacc("TRN2", target_bir_lowering=False, debug=False,
                   num_devices=N_CORES)
    d_xT = nc.dram_tensor("xT", [D, BT], BF16, kind="ExternalInput").ap()
    d_wq = nc.dram_tensor("wq", [D, 128], BF16, kind="ExternalInput").ap()
    d_wk = nc.dram_tensor("wk", [D, 128], BF16, kind="ExternalInput").ap()
    d_wv = nc.dram_tensor("wv", [D, 128], BF16, kind="ExternalInput").ap()
    d_wo = nc.dram_tensor("wo", [128, D], BF16, kind="ExternalInput").ap()
    d_bias = nc.dram_tensor("bias", [128, 3], F32, kind="ExternalInput").ap()
    d_tri = nc.dram_tensor("tri", [128, 128], BF16, kind="ExternalInput").ap()
    d_ident = nc.dram_tensor("ident", [128, 128], BF16, kind="ExternalInput").ap()
    d_out = nc.dram_tensor("out", [BT, D], F32, kind="ExternalOutput").ap()

    with tile.TileContext(nc) as tc:
        with ExitStack() as ctx:
            consts = ctx.enter_context(tc.tile_pool(name="consts", bufs=1))
            big = ctx.enter_context(tc.tile_pool(name="big", bufs=1))
            vtpool = ctx.enter_context(tc.tile_pool(name="vt", bufs=2))
            xpool = ctx.enter_context(tc.tile_pool(name="xt", bufs=3))
            ppool = ctx.enter_context(tc.tile_pool(name="pt", bufs=6))
            opool = ctx.enter_context(tc.tile_pool(name="ot", bufs=12))
            rpool = ctx.enter_context(tc.tile_pool(name="rc", bufs=2))
            outp = ctx.enter_context(tc.tile_pool(name="outp", bufs=6))
            psA = ctx.enter_context(tc.tile_pool(name="psA", bufs=2, space="PSUM"))
            psO = ctx.enter_context(tc.tile_pool(name="psO", bufs=2, space="PSUM"))
            psM = ctx.enter_context(tc.tile_pool(name="psM", bufs=1, space="PSUM"))

            # ---- constants ----
            wq_sb = consts.tile([128, 1024], BF16, tag="wq")
            wk_sb = consts.tile([128, 1024], BF16, tag="wk")
            wv_sb = consts.tile([128, 1024], BF16, tag="wv")
            # (c p) n -> p (c n): k-chunk c of W lands at cols [c*128, c*128+128)
            for w_sb, d_w in ((wq_sb, d_wq), (wk_sb, d_wk), (wv_sb, d_wv)):
                nc.sync.dma_start(
                    w_sb[:].rearrange("p (c n) -> p c n", c=8),
                    d_w.rearrange("(c p) n -> p c n", p=128))
            wo_sb = consts.tile([128, 1024], BF16, tag="wo")
            nc.sync.dma_start(wo_sb[:], d_wo[:])
            bias_sb = consts.tile([128, 3], F32, tag="bias")
            nc.sync.dma_start(bias_sb[:], d_bias[:])
            tri_sb = consts.tile([128, 128], BF16, tag="tri")
            nc.sync.dma_start(tri_sb[:], d_tri[:])
            ident_sb = consts.tile([128, 128], BF16, tag="ident")
            nc.sync.dma_start(ident_sb[:], d_ident[:])

            # ---- persistent per-batch tensors ----
            qt = [big.tile([128, T], BF16, tag=f"qt{b}", name=f"qt{b}")
                  for b in range(B)]
            kt = [big.tile([128, T], BF16, tag=f"kt{b}", name=f"kt{b}")
                  for b in range(B)]
            # v_sb[b]: 16 key chunks x [V_h0 | ones | V_h1 | ones] (256 cols)
            v_sb = [big.tile([128, NKC * 256], BF16, tag=f"v{b}", name=f"v{b}")
                    for b in range(B)]
            for b in range(B):
                nc.gpsimd.memset(v_sb[b][:], 1.0)

            vt_tmp = [None] * B   # V^T staging per batch
            proj_prog = [0] * B   # completed t-blocks per batch
            proj_cap = [TPB] * B  # phase throttle: max t-blocks to emit

            def proj_gen(b):
                """Projection + V transpose of one t-block at a time, in
                small PE steps. attn(b, qb) only reads K/V key chunks up to
                t-block qb, so later t-blocks legally interleave INTO batch
                b's own attention - the filler that keeps the last batch's
                TensorEngine dense (and its HAM clock warm)."""
                vt_tmp[b] = vtpool.tile([128, T], BF16, tag="vt", name=f"vt{b}")
                for tbl in range(TPB):
                    x_t = xpool.tile([128, 8 * 512], BF16, tag="xt",
                                     name=f"x{b}_{tbl}")
                    for c in range(8):
                        nc.sync.dma_start(
                            x_t[:, bass.ts(c, 512)],
                            d_xT[c * 128: c * 128 + 128,
                                 bass.ts(b * TPB + tbl, 512)])
                    for pi, (w_sb, col) in enumerate(
                            ((wq_sb, 0), (wk_sb, 1), (wv_sb, 2))):
                        # alternate pools: double-buffers the accumulator
                        # across groups despite each pool having bufs=1
                        gpool, gtag = ((psM, "proj"), (psO, "proj2"))[
                            (tbl * 3 + pi) % 2]
                        ps = gpool.tile([128, 512], F32, tag=gtag, bufs=1,
                                        name=f"pj{b}_{tbl}_{pi}")
                        for c in range(8):
                            nc.tensor.matmul(
                                ps[:], w_sb[:, bass.ts(c, 128)],
                                x_t[:, bass.ts(c, 512)],
                                start=(c == 0), stop=(c == 7))
                            yield
                        dest = (qt[b], kt[b], vt_tmp[b])[pi]
                        nc.vector.tensor_scalar_add(
                            dest[:, bass.ts(tbl, 512)], ps[:],
                            bias_sb[:, col:col + 1])
                    # this t-block's V^T -> v_sb [t, (V|1|V|1)]: 4 PE
                    # transposes staged in the 1-bank psM pool (bitcast)
                    tp = psM.tile([128, 512], F32, tag="proj",
                                  name=f"tp{b}_{tbl}")
                    tpb = tp[:].bitcast(BF16)
                    for t4 in range(4):
                        tc16 = tbl * 4 + t4
                        nc.tensor.transpose(
                            tpb[:, t4 * 128: t4 * 128 + 128],
                            vt_tmp[b][:, bass.ts(tc16, 128)], ident_sb[:])
                    src = bass.AP(tpb.tensor, tpb.offset,
                                  [tpb.ap[0], [128, 4], [64, 2], [1, 64]])
                    dst0 = v_sb[b][:, tbl * 1024: tbl * 1024 + 1024]
                    dst = bass.AP(dst0.tensor, dst0.offset,
                                  [dst0.ap[0], [256, 4], [128, 2], [1, 64]])
                    nc.vector.tensor_copy(dst, src)
                    proj_prog[b] = tbl + 1
                    yield

            def outproj_gen(b, qb, o_sb):
                """out[q, n] = sum_dv O^T[dv, q] * W_out[dv, n], per q-chunk."""
                for qc in range(4):
                    op = psA.tile([128, 1024], F32, tag="sA",
                                  name=f"op{b}_{qb}_{qc}")
                    for n2 in range(2):
                        nc.tensor.matmul(
                            op[:, bass.ts(n2, 512)],
                            o_sb[:, bass.ts(qc, 128)],
                            wo_sb[:, bass.ts(n2, 512)],
                            start=True, stop=True)
                        yield
                    osb = outp.tile([128, 1024], F32, tag="outp",
                                    name=f"ob{b}_{qb}_{qc}")
                    nc.vector.tensor_copy(osb[:], op[:])
                    row = b * T + qb * 512 + qc * 128
                    nc.sync.dma_start(d_out[row:row + 128, :], osb[:])
                    yield

            fill_proj = []  # (batch, generator) projection fillers
            fill_op = []    # short out-projection generators


            def pull_from(lst, n):
                for _ in range(n):
                    while lst:
                        try:
                            next(lst[0])
                            break
                        except StopIteration:
                            lst.pop(0)
                    else:
                        break

            def pull_proj(n):
                got = 0
                for _ in range(n):
                    while fill_proj:
                        pb, g = fill_proj[0]
                        if proj_prog[pb] >= proj_cap[pb]:
                            return got  # head gen throttled for a later phase
                        try:
                            next(g)
                            got += 1
                            break
                        except StopIteration:
                            fill_proj.pop(0)
                    else:
                        break
                return got

            def force_proj(b, upto):
                """Emit batch b's projection through t-block `upto` NOW
                (earlier batches' leftovers drain first - they are older
                dependencies by construction)."""
                proj_cap[b] = max(proj_cap[b], upto)
                while proj_prog[b] < upto and fill_proj:
                    pb, g = fill_proj[0]
                    try:
                        next(g)
                    except StopIteration:
                        fill_proj.pop(0)

            def pull(n):
                # out-projections are short and slot-critical: keep them moving
                pull_from(fill_op, 1)
                pull_proj(n)

            def attn(b, qb):
                """Attention for query block qb of batch b."""
                o_ps = [psO.tile([128, 512], F32, tag="o",
                                 name=f"ops{b}_{qb}_{h}") for h in range(2)]
                nch = 4 * qb + 4
                pending = None  # (p_t, off, j) awaiting AV matmuls

                def av(p_t, off, j):
                    for h in range(2):
                        lo = off if h == 0 else 512
                        nc.tensor.matmul(
                            o_ps[h][:, off:512],
                            v_sb[b][:, j * 256 + h * 128: j * 256 + h * 128 + 128],
                            p_t[:, lo: lo + 512 - off],
                            start=(j == 0), stop=(j == nch - 1))

                for j in range(nch):
                    r = j - 4 * qb
                    off = 128 * r if r >= 0 else 0
                    s_ps = psA.tile([128, 1024], F32, tag="sA",
                                    name=f"s{b}_{qb}_{j}")
                    p_t = ppool.tile([128, 1024], BF16, tag="pt",
                                     name=f"p{b}_{qb}_{j}")
                    # h0's valid q-range lands at [off:512], h1's at
                    # [512:1024-off]: adjacent, so one exp covers both heads
                    for h in range(2):
                        lo = off if h == 0 else 512
                        nc.tensor.matmul(
                            s_ps[:, lo: lo + 512 - off],
                            kt[b][64 * h: 64 * h + 64, bass.ts(j, 128)],
                            qt[b][64 * h: 64 * h + 64,
                                  qb * 512 + off: qb * 512 + 512],
                            start=True, stop=True, tile_position=(64 * h, 0))
                    nc.scalar.activation(p_t[:, off: 1024 - off],
                                         s_ps[:, off: 1024 - off],
                                         EXPF, scale=0.125)
                    if r >= 0:
                        for h in range(2):
                            lo = off if h == 0 else 512
                            nc.vector.tensor_mul(
                                p_t[:, lo: lo + 128],
                                p_t[:, lo: lo + 128],
                                tri_sb[:])
                    if pending is not None:
                        av(*pending)
                    pending = (p_t, off, j)
                    pull(2)
                av(*pending)
                # bound the out-projection backlog so the o_sb slot chain
                # below can't deadlock (opool bufs exceeds backlog + 1);
                # the backlog doubles as PE filler for the last batch,
                # which has no projection work left to interleave - but
                # taper it off through that batch so nothing piles into a
                # serial drain after the last attention block
                # out-projection backlog: batch 0 drains promptly; batches
                # 1-2 defer (reserve PE work for the filler-poor tail);
                # batch 3 spends the reserve across its blocks, ending dry
                if b == 0:
                    limit = 5
                elif b < 3:
                    limit = 99
                else:
                    limit = max(0, 6 - 2 * qb)
                while len(fill_op) > limit:
                    pull_from(fill_op, 10 ** 9)

                # normalize: O[dv, q] / denom[q] (denom replicated on 64:128)
                o_sb = opool.tile([128, 512], BF16, tag="ot",
                                  name=f"o{b}_{qb}")
                for h in range(2):
                    lg = rpool.tile([64, 512], F32, tag="lg",
                                    name=f"lg{b}_{qb}_{h}")
                    rec = rpool.tile([64, 512], F32, tag="rc",
                                     name=f"r{b}_{qb}_{h}")
                    _act_recip(nc, rec[:], o_ps[h][64:128, :], lg[:])
                    nc.vector.tensor_mul(
                        o_sb[64 * h: 64 * h + 64, :], o_ps[h][0:64, :], rec[:])
                fill_op.append(outproj_gen(b, qb, o_sb))

            # ---- emission ----
            fill_proj.append((0, proj_gen(0)))
            force_proj(0, TPB)
            for b in range(B):
                if b + 1 < B:
                    # reserve the last batch's later t-blocks as filler for
                    # its own attention; earlier batches emit fully as filler
                    # of their predecessor
                    proj_cap[b + 1] = 1 if b + 1 == B - 1 else TPB
                    fill_proj.append((b + 1, proj_gen(b + 1)))
                if b == B - 1:
                    proj_cap[b] = TPB
                for qb in range(NQB):
                    # attention of block qb reads K/V only up to t-block qb
                    force_proj(b, min(qb + 1, TPB))
                    attn(b, qb)
            pull_from(fill_op, 10 ** 9)

    nc.compile()
    return nc


def _prep_inputs(x, W_qkv, b_qkv, W_out):
    bf = ml_dtypes.bfloat16
    flat = np.ascontiguousarray(x.reshape(BT, D))
    xT = np.ascontiguousarray(flat.T).astype(bf)
    tri = np.triu(np.ones((128, 128), np.float32)).astype(bf)
    ident = np.eye(128, dtype=np.float32).astype(bf)
    in_maps = []
    for c in range(N_CORES):
        sl = slice(128 * c, 128 * c + 128)
        in_maps.append({
            "xT": xT,
            "wq": np.ascontiguousarray(W_qkv[:, 0 * D:1 * D][:, sl]).astype(bf),
            "wk": np.ascontiguousarray(W_qkv[:, 1 * D:2 * D][:, sl]).astype(bf),
            "wv": np.ascontiguousarray(W_qkv[:, 2 * D:3 * D][:, sl]).astype(bf),
            "wo": np.ascontiguousarray(W_out[sl, :]).astype(bf),
            "bias": np.ascontiguousarray(np.stack(
                [b_qkv[0 * D:1 * D][sl], b_qkv[1 * D:2 * D][sl],
                 b_qkv[2 * D:3 * D][sl]], axis=1)).astype(np.float32),
            "tri": tri,
            "ident": ident,
        })
    return in_maps


def kernel(x, W_qkv, b_qkv, W_out, b_out):
    global _CACHED_NC, LAST_RESULTS
    x = np.asarray(x, np.float32)
    W_qkv = np.asarray(W_qkv, np.float32)
    b_qkv = np.asarray(b_qkv, np.float32)
    W_out = np.asarray(W_out, np.float32)
    b_out = np.asarray(b_out, np.float32)

    if _CACHED_NC is None:
        _CACHED_NC = _build()
    in_maps = _prep_inputs(x, W_qkv, b_qkv, W_out)
    res = run_bass_kernel_spmd(
        _CACHED_NC, in_maps, core_ids=list(range(N_CORES)),
        trace=bool(int(os.environ.get("ATTN_TRACE", "0"))))
    LAST_RESULTS = res
    acc = np.zeros((BT, D), np.float64)
    for r in res.results:
        acc += r["out"].astype(np.float64)
    out = (acc + b_out.astype(np.float64)).astype(np.float32)
    return out.reshape(B, T, D)
